# revision 11
# baseline (speedup 1.0000x reference)
"""Trainium2 Bass kernel for nn_Attention_33200097198117 (optimized round 2).

B=16, N=1025, C=768, H=12 RoPE attention. Data-parallel over batch:
each of the 8 NeuronCores computes 2 batches with the full weights; the
full output is the concatenation over cores (no collectives needed).

Structure (per core, 2 batches x 2 head-groups = 4 stages):
 - prologue: hoist W_qkv/W_proj to SBUF (bf16), build xT for both batches,
   sin/cos tables; DMAs spread across sync/scalar/gpsimd queues.
 - software pipeline: while the attention heads of stage s run (paced by
   the Act-engine exp stream), the QKV projection of stage s+1 is
   interleaved into the PE stream; the output projection of batch 0 is
   interleaved into the last stage's attention.
 - per-head softmax normalization is pure DVE: v_aug carries 64 ones
   columns so PV leaves 64 replicated copies of Z in o_ps rows [64:128];
   normalize = reciprocal + multiply, nothing on PE/Act/Pool.
 - PSUM banks: scores [128,1024] x2 (4) + o_ps [128,1024] (2) +
   chunked-qkv [128,512] x2 (2) = 8.
"""

import numpy as np

# ---------------------------------------------------------------------------
# Toolchain compatibility: this container's walrus accepts at most ONE sync
# wait entry per instruction, while Tile's scheduler attaches several (and
# its kernel-tail drain collects one per outstanding semaphore). Patch the
# tail drain and post-process the module to split multi-wait instructions.
# ---------------------------------------------------------------------------
import concourse.tile as tile
from bass_rust import ScopedClock


def _drain_and_barrier(self, tick_clock, wait_clock):
    drain_inst = self.nc.sync.drain()
    wait_clock.add_sem_waits(drain_inst.ins, ScopedClock({None: tick_clock.global_clock}))
    si = drain_inst.ins.sync_info
    waits = list(si.on_wait) if si is not None else []
    if len(waits) > 1:
        si.on_wait = [waits[0]]
        assert self.sems is not None
        allocated = dict(self.sems.allocated())
        by_name = {}
        for v in allocated.values():
            by_name[getattr(v, "name", None)] = v
        for w in waits[1:]:
            sem = by_name.get(w.ant_name) or allocated.get(w.ant_name)
            assert sem is not None, f"sem {w.ant_name} not found"
            nop = self.nc.sync.nop()
            assert w.wait_mode in ("sem-ge-imm", "sem-ge"), w.wait_mode
            nop.wait_op(sem, w.wait_value, "sem-ge")

    self.nc.all_engine_barrier()
    assert self.sems is not None
    popped = self.nc._tile_sem_poison_stack.pop()
    assert popped is self._sem_poison
    self.nc.clear_and_free_semaphores(list(self.sems.allocated().values()))
    self.nc.all_engine_barrier()


tile.TileContext._drain_and_barrier = _drain_and_barrier


def split_multi_waits(nc):
    """Hoist extra sync waits onto cloned NoOps before each instruction."""
    import copy
    import bass_rust

    template = None
    for f in nc.m.functions:
        for b in f.blocks:
            for inst in b.instructions:
                if type(inst).__name__ == "InstNoOp":
                    template = inst
                    break
            if template is not None:
                break
    assert template is not None, "need one InstNoOp in module as clone template"

    for f in nc.m.functions:
        for b in f.blocks:
            changed = False
            out = []
            for inst in b.instructions:
                si = inst.sync_info
                waits = list(si.on_wait) if si is not None else []
                if len(waits) > 1:
                    changed = True
                    for i, w in enumerate(waits[:-1]):
                        n = copy.copy(template)
                        n.name = f"{inst.name}-wsplit{i}"
                        n.engine = inst.engine
                        n.sync_info = bass_rust.SyncInfo(on_wait=[w], on_update=[])
                        out.append(n)
                    si.on_wait = [waits[-1]]
                out.append(inst)
            if changed:
                b.instructions = out


from contextlib import ExitStack

import concourse.bass as bass
import concourse.mybir as mybir
import concourse.tile as tile
from concourse.masks import make_identity

F32 = mybir.dt.float32
BF16 = mybir.dt.bfloat16
AF = mybir.ActivationFunctionType

B_PER_CORE = 2
N = 1025
C = 768
H = 12
DH = 64
NT = 8          # full 128-token tiles
NPAD = 1152     # qkT free-dim allocation (1024 + 128 zero pad incl. col 1024)
SCALE = DH ** -0.5

F32R = mybir.dt.float32r


def _mm(ap):
    return ap.bitcast(F32R)


def _m32(ap):
    return ap.bitcast(F32)


def build_rot_matrix(nc, rot):
    """lhsT for rotate_half: out = rot.T @ qT gives rot(q) rows.
    rot[p, f] = +1 where f = p + 32 (p%64 < 32), -1 where f = p - 32 (p%64 >= 32),
    applied per 64-row head block (two blocks stacked)."""
    nc.gpsimd.memset(rot, 0.0)
    for blk in range(2):
        b0 = 64 * blk
        nc.gpsimd.affine_select(
            out=rot[b0:b0 + 32, :],
            in_=rot[b0:b0 + 32, :],
            compare_op=mybir.AluOpType.not_equal,
            fill=1.0,
            base=b0 + 32,
            pattern=[[-1, 128]],
            channel_multiplier=1,
        )
        nc.gpsimd.affine_select(
            out=rot[b0 + 32:b0 + 64, :],
            in_=rot[b0 + 32:b0 + 64, :],
            compare_op=mybir.AluOpType.not_equal,
            fill=-1.0,
            base=b0,
            pattern=[[-1, 128]],
            channel_multiplier=1,
        )


def build_kernel():
    nc = bass.Bass("TRN2", target_bir_lowering=False, debug=False, num_devices=8)

    x = nc.dram_tensor("x", [B_PER_CORE, N, C], F32, kind="ExternalInput").ap()
    sin = nc.dram_tensor("sin", [N - 1, DH], F32, kind="ExternalInput").ap()
    cos = nc.dram_tensor("cos", [N - 1, DH], F32, kind="ExternalInput").ap()
    w_qkv = nc.dram_tensor("w_qkv", [C, 3 * C], F32, kind="ExternalInput").ap()
    w_proj = nc.dram_tensor("w_proj", [C, C], F32, kind="ExternalInput").ap()
    b_proj = nc.dram_tensor("b_proj", [C], F32, kind="ExternalInput").ap()
    y = nc.dram_tensor("y", [B_PER_CORE, N, C], F32, kind="ExternalOutput").ap()

    with tile.TileContext(nc) as tc, ExitStack() as ctx:
        nc.sync.nop(nofuse=True)  # clone template for split_multi_waits
        const = ctx.enter_context(tc.tile_pool(name="const", bufs=1))
        big = ctx.enter_context(tc.tile_pool(name="bigbuf", bufs=1))
        pt_pool = ctx.enter_context(tc.tile_pool(name="pt", bufs=3))
        xn_pool = ctx.enter_context(tc.tile_pool(name="xn", bufs=3))
        raw_pool = ctx.enter_context(tc.tile_pool(name="raw", bufs=3))
        rcp_pool = ctx.enter_context(tc.tile_pool(name="rcp", bufs=1))
        tmp_pool = ctx.enter_context(tc.tile_pool(name="tmp", bufs=2))
        y_pool = ctx.enter_context(tc.tile_pool(name="ystage", bufs=2))
        nrm_pool = ctx.enter_context(tc.tile_pool(name="nrm", bufs=2))
        ps_sc = ctx.enter_context(tc.tile_pool(name="ps_sc", bufs=2, space="PSUM"))
        ps_o = ctx.enter_context(tc.tile_pool(name="ps_o", bufs=1, space="PSUM"))
        ps_q = ctx.enter_context(tc.tile_pool(name="ps_q", bufs=2, space="PSUM"))

        def p_score():
            return ps_sc.tile([128, 1024], F32, tag="sc", name="p_score")

        def p_ops():
            return ps_o.tile([128, 1024], F32, tag="ops", name="p_ops")

        def p_q():
            return ps_q.tile([128, 512], F32, tag="q", name="p_q")

        # ---------------- constants ----------------
        identf = const.tile([128, 128], F32, tag="identf")
        make_identity(nc, identf[:])
        ident = const.tile([128, 128], F32R, tag="ident")
        nc.vector.tensor_copy(out=ident[:], in_=identf[:])
        rotf = const.tile([128, 128], F32, tag="rotf")
        build_rot_matrix(nc, rotf[:])
        rot = const.tile([128, 128], F32R, tag="rot")
        nc.vector.tensor_copy(out=rot[:], in_=rotf[:])
        rot_bf = const.tile([128, 128], BF16, tag="rot_bf")
        nc.vector.tensor_copy(out=rot_bf[:], in_=rotf[:])
        onesf = const.tile([128, 1], F32, tag="onesf")
        nc.vector.memset(onesf[:], 1.0)
        zerof = const.tile([128, 1], F32, tag="zerof")
        nc.vector.memset(zerof[:], 0.0)

        # ---------------- hoisted weights (bf16, gpsimd DMA queue) --------
        # wsl[:, g, k]: [9, 128]; slabs 0..2 = q couts of group g, 3..5 = k
        # couts, 6..8 = v couts. c_in rows 128k..128k+127.
        wsl = big.tile([128, 2, 6, 9, 128], BF16, tag="wslab")
        # t-tile order in W_qkv cols: q g0, q g1, k g0, k g1, v g0, v g1,
        # 3 tiles each -> t = blk*6 + grp*3 + i
        wq_r = w_qkv.rearrange("c (blk grp i p) -> c blk grp i p",
                               blk=3, grp=2, i=3, p=128)

        def load_wsl_slab(g, k, cast_engine):
            r0, r1 = 128 * k, 128 * (k + 1)
            stg = tmp_pool.tile([128, 9, 128], F32, tag="wstg", name="wstg")
            nc.sync.dma_start(
                stg.rearrange("p (blk i) q -> p blk i q", blk=3),
                wq_r[r0:r1, :, g, :, :])
            if cast_engine == "act":
                nc.scalar.activation(wsl[:, g, k, :, :], stg[:], AF.Copy)
            elif cast_engine == "dve":
                nc.vector.tensor_copy(out=wsl[:, g, k, :, :], in_=stg[:])
            else:
                nc.gpsimd.tensor_copy(out=wsl[:, g, k, :, :], in_=stg[:])

        def load_wsl_g(g, cast_engine):
            for k in range(6):
                load_wsl_slab(g, k, cast_engine)

        wproj6 = big.tile([128, 6, C], BF16, tag="wproj")

        def load_wproj():
            load_bias()
            load_wsl_g(1, "dve")
            for ct in range(6):
                stg = tmp_pool.tile([128, 9, 128], F32, tag="wstg")
                nc.sync.dma_start(
                    stg[:, 0:6, :],
                    w_proj[128 * ct:128 * (ct + 1), :].rearrange("p (a b) -> p a b", b=128))
                nc.gpsimd.tensor_copy(
                    out=wproj6[:, ct, :],
                    in_=stg[:, 0:6, :].rearrange("p a b -> p (a b)"))

        # sinT/cosT: [128, N] coeff col t = (sin,cos) for token t. One DMA
        # per table; transposes run first so PE has work while x00 arrives.
        sinT = const.tile([128, N], F32, tag="sinT")
        cosT = const.tile([128, N], F32, tag="cosT")

        def build_sincos():
            nc.vector.memset(sinT[:, 0:1], 0.0)
            nc.vector.memset(cosT[:, 0:1], 1.0)
            sin_nat = sin.rearrange("(o p) d -> p o d", p=128)
            cos_nat = cos.rearrange("(o p) d -> p o d", p=128)
            for src_nat, dstT in ((sin_nat, sinT), (cos_nat, cosT)):
                nat = xn_pool.tile([128, NT, DH], F32R, tag="xnat", name="scnat")
                nc.sync.dma_start(nat[:], src_nat.bitcast(F32R))
                for t in range(NT):
                    pt = p_q()
                    nc.tensor.transpose(pt[0:DH, 0:128].bitcast(F32R),
                                        nat[:, t, :], ident[:])
                    nc.vector.tensor_copy(
                        out=dstT[0:DH, 1 + 128 * t:1 + 128 * (t + 1)],
                        in_=pt[0:DH, 0:128],
                    )
            nc.gpsimd.dma_start(sinT[64:128, :], sinT[0:64, :])
            nc.gpsimd.dma_start(cosT[64:128, :], cosT[0:64, :])

        # ---- xT: [128, 6, N] = x[b].T, bf16. Emitted as items so batch
        # 1's build can interleave into stage-0 attention. Each item
        # prefetches the next tile's DMA before transposing its own. ----
        xTs = [big.tile([128, 6, N], BF16, tag=f"xT{b}", name=f"xT{b}")
               for b in range(B_PER_CORE)]
        xnats = {}

        def xt_dma(b, t):
            xnat = xn_pool.tile([128, C], F32R, tag="xnat")
            xnats[(b, t)] = xnat
            rows = x[b, 128 * t:128 * (t + 1), :].bitcast(F32R)
            if b == 0:
                # prologue: split across two queues to double the feed rate
                nc.sync.dma_start(xnat[:, 0:384], rows[:, 0:384])
                nc.scalar.dma_start(xnat[:, 384:768], rows[:, 384:768])
            else:
                nc.sync.dma_start(xnat[:], rows)

        def xt_items(b):
            xT = xTs[b]

            def t_item(t):
                def f():
                    if t + 2 < NT:
                        xt_dma(b, t + 2)
                    xnat = xnats.pop((b, t))
                    for kk, kw in ((0, 4), (4, 2)):
                        pt = p_q()
                        for j in range(kw):
                            nc.tensor.transpose(
                                pt[:, 128 * j:128 * (j + 1)].bitcast(F32R),
                                xnat[:, 128 * (kk + j):128 * (kk + j + 1)],
                                ident[:],
                            )
                        if b == 0 and t % 2 == 1:
                            nc.scalar.activation(
                                xT[:, kk:kk + kw, 128 * t:128 * (t + 1)],
                                pt[:, 0:128 * kw].rearrange("p (a b) -> p a b", a=kw),
                                AF.Copy)
                        else:
                            nc.vector.tensor_copy(
                                out=xT[:, kk:kk + kw, 128 * t:128 * (t + 1)],
                                in_=pt[:, 0:128 * kw].rearrange("p (a b) -> p a b", a=kw),
                            )
                return f

            def tail_item():
                xtail = nrm_pool.tile([128, 8], F32, tag="xtail")
                with nc.allow_non_contiguous_dma(reason="single tail token scatter"):
                    nc.sync.dma_start(
                        xtail[:, 0:6],
                        x[b, 1024, :].rearrange("(k p) -> p k", p=128),
                    )
                nc.vector.tensor_copy(
                    out=xT[:, :, 1024:1025],
                    in_=xtail[:, 0:6].rearrange("p (a c) -> p a c", c=1))

            return [t_item(t) for t in range(NT)] + [tail_item]

        # batch 0 xT inline (stage-0 QKV needs it). sincos transposes give
        # PE work while x00 is in flight; group-0 weight slabs interleave
        # between xT items so x tiles arrive just-in-time.
        build_sincos()
        xt_dma(0, 0)
        xt_dma(0, 1)
        for t, it in enumerate(xt_items(0)):
            it()
            if 1 <= t <= 6:
                load_wsl_slab(0, t - 1, "act")


        bias_bc = const.tile([128, C], F32, tag="bias")

        def load_bias():
            nc.sync.dma_start(bias_bc[0:1, :], b_proj[None, :])
            p = 1
            while p < 128:
                nc.sync.dma_start(bias_bc[p:2 * p, :], bias_bc[0:p, :])
                p *= 2

        # ================= stage machinery =================
        _mk_state = {}
        # A stage is (b, g). make_qkv(b, g, slot) returns (qkT, v_aug,
        # items): item thunks that emit the QKV projection piecewise so they
        # can be interleaved into the previous stage's attention heads.

        def make_qkv(b, g, slot, wide=False):
            xT = xTs[b]
            qkT = big.tile([128, 6, NPAD], BF16, tag=f"qkT{slot}")
            v_aug = big.tile([128, NT + 1, 6, 128], BF16, tag=f"vaug{slot}")
            items = []
            raws = {}
            state = {"last_m": None}

            def init_item():
                nc.vector.memset(v_aug[:, 0:NT, :, DH:128], 1.0)
            items.append(init_item)

            def rope_flush():
                if state["last_m"] is not None:
                    rope_finish(state["last_m"])
                    state["last_m"] = None

            def rope_finish(m):
                raw = raws.pop(m)
                for c0 in (0, 512):
                    rp = p_q()
                    nc.tensor.matmul(
                        rp[:, 0:512],
                        lhsT=rot[:],
                        rhs=_mm(raw[:, c0:c0 + 512]),
                        start=True, stop=True,
                    )
                    t1 = raw_pool.tile([128, 512], F32, tag="ropet1")
                    nc.vector.tensor_tensor(
                        t1[:], rp[:, 0:512], sinT[:, c0:c0 + 512],
                        mybir.AluOpType.mult)
                    nc.vector.tensor_tensor(
                        raw[:, c0:c0 + 512], raw[:, c0:c0 + 512],
                        cosT[:, c0:c0 + 512], mybir.AluOpType.mult)
                    nc.gpsimd.tensor_tensor(
                        qkT[:, m, c0:c0 + 512], t1[:], _m32(raw[:, c0:c0 + 512]),
                        mybir.AluOpType.add)

            def m_item(m, wide_item=None):
                use_wide = wide if wide_item is None else wide_item
                def f():
                    raw = raw_pool.tile([128, 1024], F32R, tag="qkraw")
                    raws[m] = raw
                    if use_wide:
                        # scores psum is idle during the solo stage-0 QKV:
                        # use a wide tile for a deep 12-matmul accumulation
                        qp = p_score()
                        for k in range(6):
                            for c0 in (0, 512):
                                nc.tensor.matmul(
                                    qp[:, c0:c0 + 512],
                                    lhsT=wsl[:, g, k, m, :],
                                    rhs=xT[:, k, c0:c0 + 512],
                                    start=(k == 0), stop=(k == 5),
                                )
                        nc.scalar.activation(raw[:], qp[:, 0:1024], AF.Copy)
                    else:
                        for c0 in (0, 512):
                            qp = p_q()
                            for k in range(6):
                                nc.tensor.matmul(
                                    qp[:, 0:512],
                                    lhsT=wsl[:, g, k, m, :],
                                    rhs=xT[:, k, c0:c0 + 512],
                                    start=(k == 0), stop=(k == 5),
                                )
                            nc.scalar.activation(raw[:, c0:c0 + 512], qp[:, 0:512],
                                                 AF.Copy)
                    prev, state["last_m"] = state["last_m"], m
                    if prev is not None:
                        rope_finish(prev)
                return f
            for m in range(6):
                items.append(m_item(m))

            def v_item(t):
                def f():
                    if t == 0:
                        rope_flush()
                    vp = p_q()
                    for k in range(6):
                        nc.tensor.matmul(
                            vp[:, 0:384],
                            lhsT=xT[:, k, 128 * t:128 * (t + 1)],
                            rhs=wsl[:, g, k, 6:9, :],
                            start=(k == 0), stop=(k == 5),
                        )
                    nc.vector.tensor_copy(
                        out=v_aug[:, t, :, 0:DH],
                        in_=vp[:, 0:384].rearrange("p (a b) -> p a b", a=6),
                    )
                return f
            for t in range(NT):
                items.append(v_item(t))

            def tail_item():
                # row-form qkv for token 1024
                tail_qk_sb = const.tile([1, 768], F32R, tag="tailqksb")
                tail_v = p_q()
                for c0, cw in ((0, 512), (512, 256)):
                    tq = p_q()
                    for k in range(6):
                        nc.tensor.matmul(
                            tq[0:1, 0:cw],
                            lhsT=xT[:, k, 1024:1025],
                            rhs=wsl[:, g, k, 0:6, :].rearrange(
                                "p a b -> p (a b)")[:, c0:c0 + cw],
                            start=(k == 0), stop=(k == 5),
                        )
                    nc.vector.tensor_copy(
                        out=tail_qk_sb[0:1, c0:c0 + cw], in_=tq[0:1, 0:cw])
                for k in range(6):
                    nc.tensor.matmul(
                        tail_v[0:1, 0:384],
                        lhsT=xT[:, k, 1024:1025],
                        rhs=wsl[:, g, k, 6:9, :],
                        start=(k == 0), stop=(k == 5),
                    )

                # tail v tile: zero everything, then write row 0 (v + ones)
                nc.vector.tensor_copy(
                    out=v_aug[:, NT, :, :],
                    in_=zerof[:, 0:1].to_broadcast([128, 6, 128]))
                nc.vector.tensor_copy(
                    out=v_aug[0:1, NT, :, 0:DH],
                    in_=tail_v[0:1, 0:384].rearrange("p (a b) -> p a b", a=6),
                )
                nc.vector.tensor_copy(
                    out=v_aug[0:1, NT, :, DH:128],
                    in_=onesf[0:1, 0:1].to_broadcast([1, 6, 64]))

                # tail qk into column layout via PE transposes
                tqp = p_q()
                for t in range(6):
                    nc.tensor.transpose(
                        tqp[:, t:t + 1],
                        tail_qk_sb[0:1, 128 * t:128 * (t + 1)].bitcast(F32),
                        ident[0:1, 0:1].bitcast(F32))
                nc.vector.tensor_copy(
                    out=qkT[:, 0:6, 1024:1025],
                    in_=tqp[:, 0:6].rearrange("p (a b) -> p a b", b=1))

                # RoPE on tail column (all 6 tiles at once)
                rp = p_q()
                nc.tensor.matmul(
                    rp[:, 0:6],
                    lhsT=rot_bf[:],
                    rhs=qkT[:, 0:6, 1024:1025],
                    start=True, stop=True,
                )
                tt1 = nrm_pool.tile([128, 6], F32, tag="tail1")
                nc.vector.tensor_tensor(
                    tt1[:], rp[:, 0:6],
                    sinT[:, 1024:1025].to_broadcast([128, 6]),
                    mybir.AluOpType.mult)
                tt2 = nrm_pool.tile([128, 6], F32, tag="tail2")
                nc.vector.tensor_tensor(
                    tt2[:], qkT[:, 0:6, 1024:1025],
                    cosT[:, 1024:1025].to_broadcast([128, 6, 1]),
                    mybir.AluOpType.mult)
                nc.vector.tensor_tensor(
                    qkT[:, 0:6, 1024:1025],
                    tt1[:].rearrange("p (a b) -> p a b", b=1),
                    tt2[:].rearrange("p (a b) -> p a b", b=1),
                    mybir.AluOpType.add)

                # zero the padding key columns [1025, NPAD)
                nc.vector.tensor_copy(
                    out=qkT[:, :, 1025:NPAD],
                    in_=zerof[:, 0:1].to_broadcast([128, 6, NPAD - 1025]))
            items.append(tail_item)
            _mk_state[slot] = (m_item, rope_flush)
            # items layout: [init, m0..m5, v0..v7, tail]
            return qkT, v_aug, items

        def attn_head(qkT, v_aug, attn_outT, g, hh, fill=None):
            pair, half = hh // 2, hh % 2
            r0 = 64 * half
            qh = qkT[r0:r0 + 64, pair, :]
            kh = qkT[r0:r0 + 64, 3 + pair, :]
            vh_t = lambda t: v_aug[:, t, hh, :]

            o_ps = p_ops()  # [128, 1024]; rows 64:128 = Z replicated
            pts = [None] * (NT + 1)

            def emit_scores(jt):
                sp = p_score()
                for c0 in (0, 512):
                    nc.tensor.matmul(
                        sp[:, c0:c0 + 512],
                        lhsT=kh[:, 128 * jt:128 * (jt + 1)],
                        rhs=qh[:, c0:c0 + 512],
                        start=True, stop=True,
                    )
                ptile = pt_pool.tile([128, 1024], BF16, tag="pt")
                pts[jt] = ptile
                nc.scalar.activation(ptile[:], sp[:, 0:1024], AF.Exp, scale=SCALE)

            def emit_pv(jt):
                for c0 in (0, 512):
                    nc.tensor.matmul(
                        o_ps[0:128, c0:c0 + 512],
                        lhsT=vh_t(jt),
                        rhs=pts[jt][:, c0:c0 + 512],
                        start=(jt == 0), stop=(jt == NT),
                    )
                pts[jt] = None

            emit_scores(0)
            for jt in range(1, NT + 1):
                if fill is not None and jt in (4, 7):
                    fill()
                emit_scores(jt)
                emit_pv(jt - 1)
            emit_pv(NT)

            # stripe B: query token 1024
            sb = p_q()  # [128, 9] scores vs tail query
            for jt in range(NT + 1):
                nc.tensor.matmul(
                    sb[:, jt:jt + 1],
                    lhsT=kh[:, 128 * jt:128 * (jt + 1)],
                    rhs=qh[:, 1024:1025],
                    start=True, stop=True,
                )
            ptb = nrm_pool.tile([128, 16], BF16, tag="ptb")
            nc.scalar.activation(ptb[:, 0:NT + 1], sb[:, 0:NT + 1],
                                 AF.Exp, scale=SCALE)
            ob = p_q()  # [128, 1]; rows 64:128 = Z_tail copies
            for jt in range(NT + 1):
                nc.tensor.matmul(
                    ob[0:128, 0:1],
                    lhsT=vh_t(jt),
                    rhs=ptb[:, jt:jt + 1],
                    start=(jt == 0), stop=(jt == NT),
                )

            # normalize: DVE reciprocal of the replicated Z rows + multiply
            h_glob = 6 * g + hh
            drow = 64 * (h_glob % 2)
            dtile = h_glob // 2
            rcp = rcp_pool.tile([64, 1032], F32, tag="rcp")
            nc.vector.reciprocal(
                out=rcp[0:64, 0:1024], in_=o_ps[DH:128, 0:1024])
            nc.vector.reciprocal(
                out=rcp[0:64, 1024:1025], in_=ob[DH:128, 0:1])
            nc.vector.tensor_tensor(
                attn_outT[drow:drow + 64, dtile, 0:1024],
                o_ps[0:DH, 0:1024], rcp[0:64, 0:1024],
                mybir.AluOpType.mult)
            nc.vector.tensor_tensor(
                attn_outT[drow:drow + 64, dtile, 1024:1025],
                ob[0:DH, 0:1], rcp[0:64, 1024:1025],
                mybir.AluOpType.mult)

        def proj_items(b, attn_outT, wide=False):
            items = []

            def i_item(it):
                def f():
                    ydst_rows = 128 if it < NT else 1
                    ysb = y_pool.tile([128, C], F32, tag="ysb")
                    if wide:
                        yp = p_score()
                        for ct in range(6):
                            for c0, cw in ((0, 512), (512, 256)):
                                nc.tensor.matmul(
                                    yp[0:ydst_rows, c0:c0 + cw],
                                    lhsT=attn_outT[:, ct, 128 * it:128 * it + ydst_rows],
                                    rhs=wproj6[:, ct, c0:c0 + cw],
                                    start=(ct == 0), stop=(ct == 5),
                                )
                        nc.vector.tensor_tensor(
                            ysb[0:ydst_rows, :], yp[0:ydst_rows, 0:C],
                            bias_bc[0:ydst_rows, :], mybir.AluOpType.add)
                    else:
                        for c0, cw in ((0, 512), (512, 256)):
                            yp = p_q()
                            for ct in range(6):
                                nc.tensor.matmul(
                                    yp[0:ydst_rows, 0:cw],
                                    lhsT=attn_outT[:, ct, 128 * it:128 * it + ydst_rows],
                                    rhs=wproj6[:, ct, c0:c0 + cw],
                                    start=(ct == 0), stop=(ct == 5),
                                )
                            nc.vector.tensor_tensor(
                                ysb[0:ydst_rows, c0:c0 + cw], yp[0:ydst_rows, 0:cw],
                                bias_bc[0:ydst_rows, c0:c0 + cw], mybir.AluOpType.add)
                    nc.sync.dma_start(
                        y[b, 128 * it:128 * it + ydst_rows, :], ysb[0:ydst_rows, :])
                return f
            for it in range(NT + 1):
                items.append(i_item(it))
            return items

        # ================= pipeline schedule =================
        stages = [(0, 0), (0, 1), (1, 0), (1, 1)]
        attn_outTs = {}
        for b in range(B_PER_CORE):
            attn_outTs[b] = big.tile([128, 6, N], BF16, tag=f"attn_outT{b}",
                                     name=f"attn_outT{b}")

        # stage 0 QKV: only what heads 0-1 need runs solo (init, m0, m3,
        # v tiles, tail); m1/m4/m2/m5 are deferred into the attention
        # interleave (heads 2+ need them, giving ~2 heads of slack).
        cur_qkT, cur_vaug, items0 = make_qkv(0, 0, slot=0, wide=True)
        init0, vs0, tail0 = items0[0], items0[7:15], items0[15]
        mk0, flush0 = _mk_state[0]
        init0()
        mk0(0, True)()
        mk0(3, True)()
        for it in vs0:
            it()
        tail0()
        deferred0 = [mk0(1, False), mk0(4, False), mk0(2, False),
                     mk0(5, False), flush0]
        xt_dma(1, 0)
        xt_dma(1, 1)
        load_wproj()

        for si, (b, g) in enumerate(stages):
            # next-stage work to interleave into this stage's attention
            if si + 1 < len(stages):
                nb, ng = stages[si + 1]
                nqkT, nvaug, nitems = make_qkv(nb, ng, slot=(si + 1) % 2)
            else:
                nqkT = nvaug = None
                nitems = proj_items(0, attn_outTs[0])
            if si == 0:
                xti = xt_items(1)
                qi = list(nitems)
                mixed = []
                while xti or qi:
                    if qi:
                        mixed.append(qi.pop(0))
                    if xti:
                        mixed.append(xti.pop(0))
                nitems = deferred0 + mixed

            pending = list(nitems)
            if len(pending) < 12:
                # sparse stage: stretch with no-ops so the work spreads
                # across all heads instead of front-loading
                stretched = []
                gaps = 12 - len(pending)
                for i, it in enumerate(pending):
                    stretched.append(it)
                    if i * gaps // max(1, len(pending)) != (i + 1) * gaps // max(1, len(pending)):
                        stretched.append(None)
                pending = stretched
            per_head = max(0, (len(pending) + 5) // 6 - 2)

            def fill():
                if pending:
                    it = pending.pop(0)
                    if it is not None:
                        it()

            for hh in range(6):
                attn_head(cur_qkT, cur_vaug, attn_outTs[b], g, hh, fill=fill)
                for _ in range(per_head):
                    fill()
            while pending:
                fill()
            del pending[:]
            cur_qkT, cur_vaug = nqkT, nvaug

        # batch-1 output projection (tail of the pipeline, scores psum idle)
        for it in proj_items(1, attn_outTs[1], wide=True):
            it()

    split_multi_waits(nc)
    return nc


_CACHED = {}


def kernel(**inputs) -> np.ndarray:
    from concourse.bass_utils import run_bass_kernel_spmd

    x = np.ascontiguousarray(np.asarray(inputs["x"], dtype=np.float32))
    B = x.shape[0]
    n_cores = 8
    per = B // n_cores
    if "nc" not in _CACHED:
        _CACHED["nc"] = build_kernel()
    nc = _CACHED["nc"]
    in_maps = []
    for c in range(n_cores):
        in_maps.append({
            "x": np.ascontiguousarray(x[c * per:(c + 1) * per]),
            "sin": np.ascontiguousarray(np.asarray(inputs["sin"], np.float32)),
            "cos": np.ascontiguousarray(np.asarray(inputs["cos"], np.float32)),
            "w_qkv": np.ascontiguousarray(np.asarray(inputs["W_qkv"], np.float32)),
            "w_proj": np.ascontiguousarray(np.asarray(inputs["W_proj"], np.float32)),
            "b_proj": np.ascontiguousarray(np.asarray(inputs["b_proj"], np.float32)),
        })
    res = run_bass_kernel_spmd(nc, in_maps, core_ids=list(range(n_cores)))
    return np.concatenate([res.results[c]["y"] for c in range(n_cores)], axis=0)
